# revision 13
# baseline (speedup 1.0000x reference)
"""Bass/Tile TRN2 kernel for 2-layer tanh RNN (B=64, T=2048, F=256, H=512).

Sharding: batch 64 -> 8 cores x 8 rows. Each core runs both layers for its
batch slice. Hidden state is kept transposed ([H partitions, batch cols]) so
each timestep is 16 fp16 128x128 matmuls accumulating into PSUM on top of a
precomputed input projection; one tanh ACT per step reads 4 PSUM banks and
writes the fp16 history strip that is also the next step's matmul operand.
Layer 1 subchunk s runs interleaved with layer 0 subchunk s+1 to hide
tanh/semaphore latency. x is cast+transposed on-chip via DMA XBAR transpose.
"""

import numpy as np

B, T, F, H, L = 64, 2048, 256, 512, 2
NCORES = 8
BL = B // NCORES            # 8 batch rows per core
CS = 32                     # timesteps per subchunk
COLS = CS * BL              # 256 psum cols per subchunk
KF = F // 128               # 2 k-chunks for layer-0 input proj
KH = H // 128               # 4 k-chunks for hidden
M = H // 128                # 4 m-chunks of output hidden

F16 = "float16"


def build_nc(Tb):
    import concourse.bacc as bacc
    import concourse.mybir as mybir
    from concourse.tile import TileContext
    from concourse.bass import ds, ts

    f32 = mybir.dt.float32
    f16 = mybir.dt.float16
    TANH = mybir.ActivationFunctionType.Tanh
    COPY = mybir.ActivationFunctionType.Copy

    S = Tb // CS            # subchunks
    NB = S // 2             # loop bodies (2 subchunks each)
    assert Tb % (2 * CS) == 0

    nc = bacc.Bacc("TRN2", target_bir_lowering=False, debug=False,
                   num_devices=NCORES)

    xr = nc.dram_tensor("xr", [BL, Tb, F], f32, kind="ExternalInput").ap()
    wihT0 = nc.dram_tensor("wihT0", [KF, 128, H], f16, kind="ExternalInput").ap()
    whhT0 = nc.dram_tensor("whhT0", [KH, 128, H], f16, kind="ExternalInput").ap()
    wihT1 = nc.dram_tensor("wihT1", [KH, 128, H], f16, kind="ExternalInput").ap()
    whhT1 = nc.dram_tensor("whhT1", [KH, 128, H], f16, kind="ExternalInput").ap()
    bT0 = nc.dram_tensor("bT0", [1, H], f16, kind="ExternalInput").ap()
    bT1 = nc.dram_tensor("bT1", [1, H], f16, kind="ExternalInput").ap()
    hTi = nc.dram_tensor("hTi", [L, KH, 128, BL], f16, kind="ExternalInput").ap()
    onesv = nc.dram_tensor("onesv", [1, COLS], f16, kind="ExternalInput").ap()
    outr = nc.dram_tensor("outr", [BL, Tb, H], f32, kind="ExternalOutput").ap()
    hlastT = nc.dram_tensor("hlastT", [L, KH, 128, BL], f32,
                            kind="ExternalOutput").ap()

    with TileContext(nc) as tc:
        with (
            tc.tile_pool(name="wpool", bufs=1) as wpool,
            tc.tile_pool(name="xtpool", bufs=1) as xtpool,
            tc.tile_pool(name="hpool", bufs=1) as hpool,
            tc.tile_pool(name="opool", bufs=1) as opool,
            tc.tile_pool(name="stage", bufs=4) as stage,
            tc.tile_pool(name="psum", bufs=1, space="PSUM") as pp,
        ):
            # ---- static tiles
            w0i = wpool.tile([128, KF, H], f16, tag="w0i")
            w0h = wpool.tile([128, KH, H], f16, tag="w0h")
            w1i = wpool.tile([128, KH, H], f16, tag="w1i")
            w1h = wpool.tile([128, KH, H], f16, tag="w1h")
            bt0 = wpool.tile([1, H], f16, tag="bt0")
            bt1 = wpool.tile([1, H], f16, tag="bt1")
            ones = wpool.tile([1, COLS], f16, tag="ones")
            hti = wpool.tile([128, L, KH, BL], f16, tag="hti")

            xT = xtpool.tile([128, KF, Tb * BL], f16, tag="xT")
            # history strips: [parity, kchunk, t, b]
            hist0 = hpool.tile([128, 2, KH, CS, BL], f16, tag="hist0")
            hist1 = hpool.tile([128, 2, KH, CS, BL], f16, tag="hist1")
            on16 = opool.tile([128, 4, H], f16, tag="on16")
            of32 = opool.tile([128, 4, H], f32, tag="of32")
            hlast = opool.tile([128, L, KH, BL], f32, tag="hlast")

            ps0 = pp.tile([128, M, 512], f32, tag="ps0")
            ps1 = pp.tile([128, M, 512], f32, tag="ps1")
            ps = [ps0, ps1]

            # ---- weight / const loads
            for k in range(KF):
                nc.sync.dma_start(out=w0i[:, k, :], in_=wihT0[k])
            for k in range(KH):
                nc.sync.dma_start(out=w0h[:, k, :], in_=whhT0[k])
                nc.sync.dma_start(out=w1i[:, k, :], in_=wihT1[k])
                nc.sync.dma_start(out=w1h[:, k, :], in_=whhT1[k])
            nc.sync.dma_start(out=bt0[:], in_=bT0[:])
            nc.sync.dma_start(out=bt1[:], in_=bT1[:])
            nc.sync.dma_start(out=ones[:], in_=onesv[:])
            for l in range(L):
                for k in range(KH):
                    nc.sync.dma_start(out=hti[:, l, k, :], in_=hTi[l, k])

            # ---- prepass: x -> fp16 -> transposed resident xT
            for blk in range(Tb // 16):
                xs = stage.tile([128, F], f32, tag="xs")
                xs16 = stage.tile([128, F], f16, tag="xs16")
                src = xr[:, ts(blk, 16), :].rearrange("b t f -> t b f")
                nc.sync.dma_start(out=xs[:], in_=src)
                if blk % 2 == 0:
                    nc.vector.tensor_copy(out=xs16[:], in_=xs[:])
                else:
                    nc.scalar.activation(xs16[:], xs[:], COPY)
                for fc in range(KF):
                    nc.sync.dma_start_transpose(
                        out=xT[:, fc, ds(blk * 128, 128)],
                        in_=xs16[:, ts(fc, 128)])

            whT = [w0h, w1h]
            hist = [hist0, hist1]
            bts = [bt0, bt1]

            def xproj(l, s, half, col0):
                """input projection + bias for layer l subchunk s into psum half"""
                off = half * COLS
                p = ps[l]
                for m in range(M):
                    nc.tensor.matmul(p[:, m, off:off + COLS],
                                     lhsT=bts[l][:, ts(m, 128)],
                                     rhs=ones[:, :COLS],
                                     start=True, stop=False,
                                     skip_group_check=True)
                if l == 0:
                    for k in range(KF):
                        rhs = xT[:, k, ds(col0, COLS)]
                        for m in range(M):
                            nc.tensor.matmul(p[:, m, off:off + COLS],
                                             lhsT=w0i[:, k, ts(m, 128)],
                                             rhs=rhs, start=False, stop=False,
                                             skip_group_check=True)
                else:
                    par = s % 2
                    for k in range(KH):
                        rhs = hist0[:, par, k, :, :]
                        for m in range(M):
                            nc.tensor.matmul(p[:, m, off:off + COLS],
                                             lhsT=w1i[:, k, ts(m, 128)],
                                             rhs=rhs, start=False, stop=False,
                                             skip_group_check=True)

            def recur_step(l, s, half, t, first):
                """one timestep of the recurrence: 16 MMs + 1 tanh"""
                par = s % 2
                p = ps[l]
                h = hist[l]
                off = half * COLS + t * BL
                for m in range(M):
                    for k in range(KH):
                        if t == 0:
                            rhs = (hti[:, l, k, :] if first
                                   else h[:, 1 - par, k, CS - 1, :])
                        else:
                            rhs = h[:, par, k, t - 1, :]
                        nc.tensor.matmul(p[:, m, off:off + BL],
                                         lhsT=whT[l][:, k, ts(m, 128)],
                                         rhs=rhs, start=False, stop=(k == KH - 1),
                                         skip_group_check=True)
                nc.scalar.activation(h[:, par, :, t, :], p[:, :, off:off + BL],
                                     TANH)

            def recur_unit(l, s, half, first=False):
                return [lambda t=t: recur_step(l, s, half, t, first)
                        for t in range(CS)]

            def interleave(*units):
                for t in range(CS):
                    for u in units:
                        u[t]()

            def out_unit(s, t0):
                """ship layer-1 subchunk s (starting at global step t0) to DRAM"""
                par = s % 2
                for g in range(2):
                    slot = par * 2 + g
                    for c in range(KH):
                        nc.sync.dma_start_transpose(
                            out=on16[:, slot, ts(c, 128)],
                            in_=hist1[:, par, c, ts(g, 16), :])
                    if g == 0:
                        nc.vector.tensor_copy(out=of32[:, slot, :],
                                              in_=on16[:, slot, :])
                    else:
                        nc.scalar.activation(of32[:, slot, :], on16[:, slot, :],
                                             COPY)
                    dst = outr[:, ds(t0 + g * 16, 16), :].rearrange(
                        "b t h -> t b h")
                    nc.sync.dma_start(out=dst, in_=of32[:, slot, :])

            # ---- peel: L0(0); then L0(1) || L1(0)
            xproj(0, 0, 0, 0)
            interleave(recur_unit(0, 0, 0, first=True))
            xproj(0, 1, 1, COLS)
            xproj(1, 0, 0, None)
            interleave(recur_unit(0, 1, 1),
                       recur_unit(1, 0, 0, first=True))
            out_unit(0, 0)

            # ---- main loop: body i handles L0(2i),L1(2i-1),L0(2i+1),L1(2i)
            if NB > 1:
                with tc.For_i(1, NB) as i:
                    col_even = i * (2 * COLS)          # xT col for s=2i
                    t_odd = i * (2 * CS) + (-CS)       # global t0 for s=2i-1
                    t_even = i * (2 * CS)              # global t0 for s=2i
                    xproj(0, 0, 0, col_even)           # s=2i  (parity 0)
                    xproj(1, 1, 1, None)               # s=2i-1 (parity 1)
                    interleave(recur_unit(0, 0, 0),    # L0(2i)
                               recur_unit(1, 1, 1))    # L1(2i-1)
                    out_unit(1, t_odd)
                    xproj(0, 1, 1, col_even + COLS)    # s=2i+1 (parity 1)
                    xproj(1, 0, 0, None)               # s=2i   (parity 0)
                    interleave(recur_unit(0, 1, 1),    # L0(2i+1)
                               recur_unit(1, 0, 0))    # L1(2i)
                    out_unit(0, t_even)

            # ---- tail: L1(S-1)
            xproj(1, S - 1, 1, None)
            interleave(recur_unit(1, S - 1, 1))
            out_unit(S - 1, Tb - CS)

            # ---- final hidden states
            fpar = (S - 1) % 2
            for l in range(L):
                nc.scalar.activation(hlast[:, l, :, :],
                                     hist[l][:, fpar, :, CS - 1, :], COPY)
                dst = hlastT[l].rearrange("c p b -> p c b")
                nc.sync.dma_start(out=dst, in_=hlast[:, l, :, :])

    nc.compile()
    return nc


_CACHE = {}
_RUNNERS = {}


class Runner:
    """jit-once PJRT runner for the compiled Bass module (8-core SPMD)."""

    def __init__(self, nc, n_cores=NCORES):
        import jax
        import numpy as _np
        from jax.sharding import Mesh, PartitionSpec
        from jax.experimental.shard_map import shard_map
        from concourse import bass2jax
        import concourse.mybir as mybir

        bass2jax.install_neuronx_cc_hook()
        self.n_cores = n_cores
        partition_name = (nc.partition_id_tensor.name
                          if nc.partition_id_tensor else None)
        in_names, out_names, out_avals, zero_shapes = [], [], [], []
        for alloc in nc.m.functions[0].allocations:
            if not isinstance(alloc, mybir.MemoryLocationSet):
                continue
            name = alloc.memorylocations[0].name
            if alloc.kind == "ExternalInput":
                if name != partition_name:
                    in_names.append(name)
            elif alloc.kind == "ExternalOutput":
                shape = tuple(alloc.tensor_shape)
                dtype = mybir.dt.np(alloc.dtype)
                out_names.append(name)
                out_avals.append(jax.core.ShapedArray(shape, dtype))
                zero_shapes.append((shape, dtype))
        self.n_params = len(in_names)
        self.in_names = list(in_names)
        self.out_names = out_names
        self.out_avals = out_avals
        self.zero_shapes = zero_shapes
        all_in = in_names + out_names + (
            [partition_name] if partition_name else [])

        def _body(*args):
            operands = list(args)
            if partition_name is not None:
                operands.append(bass2jax.partition_id_tensor())
            outs = bass2jax._bass_exec_p.bind(
                *operands,
                out_avals=tuple(out_avals),
                in_names=tuple(all_in),
                out_names=tuple(out_names),
                lowering_input_output_aliases=(),
                sim_require_finite=True,
                sim_require_nnan=True,
                nc=nc,
            )
            return tuple(outs)

        devices = jax.devices()[:n_cores]
        self.mesh = Mesh(_np.asarray(devices), ("core",))
        n_out = len(out_names)
        in_specs = (PartitionSpec("core"),) * (self.n_params + n_out)
        out_specs = (PartitionSpec("core"),) * n_out
        self.donate = tuple(range(self.n_params, self.n_params + n_out))
        self.fn = jax.jit(
            shard_map(_body, mesh=self.mesh, in_specs=in_specs,
                      out_specs=out_specs, check_rep=False),
            donate_argnums=self.donate, keep_unused=True)

    def concat_inputs(self, in_maps):
        return [np.concatenate([np.asarray(m[name]) for m in in_maps], 0)
                for name in self.in_names]

    def fresh_zeros(self):
        return [np.zeros((self.n_cores * s[0], *s[1:]), d)
                for s, d in self.zero_shapes]

    def call(self, concat_in, zeros):
        import jax
        outs = self.fn(*concat_in, *zeros)
        jax.block_until_ready(outs)
        return outs

    def run(self, in_maps):
        outs = self.call(self.concat_inputs(in_maps), self.fresh_zeros())
        res = []
        for c in range(self.n_cores):
            res.append({
                name: np.asarray(outs[i]).reshape(
                    self.n_cores, *self.out_avals[i].shape)[c]
                for i, name in enumerate(self.out_names)})
        return res


def get_runner(Tb=T):
    if Tb not in _RUNNERS:
        if Tb not in _CACHE:
            _CACHE[Tb] = build_nc(Tb)
        _RUNNERS[Tb] = Runner(_CACHE[Tb])
    return _RUNNERS[Tb]


def make_in_maps(x, h, Wih0, Whh0, b0, Wih1, Whh1, b1):
    com = _prep_weights(np.asarray(Wih0), np.asarray(Whh0), np.asarray(b0),
                        np.asarray(Wih1), np.asarray(Whh1), np.asarray(b1))
    h = np.asarray(h)
    in_maps = []
    for cidx in range(NCORES):
        bsl = slice(cidx * BL, (cidx + 1) * BL)
        hT = np.ascontiguousarray(
            h[:, bsl, :].transpose(0, 2, 1).astype(np.float16)
            .reshape(L, KH, 128, BL))
        m = dict(com)
        m["xr"] = np.ascontiguousarray(np.asarray(x)[bsl])
        m["hTi"] = hT
        in_maps.append(m)
    return in_maps


def postprocess(results, c):
    outs = np.concatenate([r["outr"] for r in results], 0)
    hparts = [r["hlastT"].transpose(0, 3, 1, 2).reshape(L, BL, H)
              for r in results]
    h_out = np.concatenate(hparts, 1)
    return outs, h_out, c


def _prep_weights(Wih0, Whh0, b0, Wih1, Whh1, b1):
    def t16(w, kc):
        # W [out, in] -> W.T [in, out] -> [kc, 128, out]
        return np.ascontiguousarray(
            w.T.astype(np.float16).reshape(kc, 128, w.shape[0]))
    com = {
        "wihT0": t16(Wih0, KF),
        "whhT0": t16(Whh0, KH),
        "wihT1": t16(Wih1, KH),
        "whhT1": t16(Whh1, KH),
        "bT0": b0.astype(np.float16).reshape(1, H),
        "bT1": b1.astype(np.float16).reshape(1, H),
        "onesv": np.ones((1, COLS), np.float16),
    }
    return com


def kernel(x, h, c, Wih0, Whh0, b0, Wih1, Whh1, b1, Tb=None):
    x = np.asarray(x)
    c = np.asarray(c)
    Tb = Tb or x.shape[1]
    runner = get_runner(Tb)
    in_maps = make_in_maps(x, h, Wih0, Whh0, b0, Wih1, Whh1, b1)
    results = runner.run(in_maps)
    return postprocess(results, c)


# revision 19
# speedup vs baseline: 22.3877x; 22.3877x over previous
"""Bass/Tile TRN2 kernel for 2-layer tanh RNN (B=64, T=2048, F=256, H=512).

Sharding: batch 64 -> 8 cores x 8 rows. Each core runs both layers for its
batch slice. Hidden state is kept transposed ([H partitions, batch cols]) so
each timestep is 16 fp16 128x128 matmuls accumulating into PSUM on top of a
precomputed input projection; one tanh ACT per step reads 4 PSUM banks and
writes the fp16 history strip that is also the next step's matmul operand.
Layer 1 subchunk s runs interleaved with layer 0 subchunk s+1 to hide
tanh/semaphore latency. x is cast+transposed on-chip via DMA XBAR transpose.
"""

import numpy as np

B, T, F, H, L = 64, 2048, 256, 512, 2
NCORES = 8
BL = B // NCORES            # 8 batch rows per core
CS = 32                     # timesteps per subchunk
COLS = CS * BL              # 256 psum cols per subchunk
KF = F // 128               # 2 k-chunks for layer-0 input proj
KH = H // 128               # 4 k-chunks for hidden
M = H // 128                # 4 m-chunks of output hidden

F16 = "float16"


def build_nc(Tb, repeat=1, col_tile=False):
    import concourse.bacc as bacc
    import concourse.mybir as mybir
    from concourse.tile import TileContext
    from concourse.bass import ds, ts

    f32 = mybir.dt.float32
    f16 = mybir.dt.float16
    TANH = mybir.ActivationFunctionType.Tanh
    COPY = mybir.ActivationFunctionType.Copy

    S = Tb // CS            # subchunks
    NB = S // 2             # loop bodies (2 subchunks each)
    assert Tb % (2 * CS) == 0

    nc = bacc.Bacc("TRN2", target_bir_lowering=False, debug=False,
                   num_devices=NCORES)

    xr = nc.dram_tensor("xr", [BL, Tb, F], f32, kind="ExternalInput").ap()
    wihT0 = nc.dram_tensor("wihT0", [KF, 128, H], f16, kind="ExternalInput").ap()
    whhT0 = nc.dram_tensor("whhT0", [KH, 128, H], f16, kind="ExternalInput").ap()
    wihT1 = nc.dram_tensor("wihT1", [KH, 128, H], f16, kind="ExternalInput").ap()
    whhT1 = nc.dram_tensor("whhT1", [KH, 128, H], f16, kind="ExternalInput").ap()
    bT0 = nc.dram_tensor("bT0", [1, H], f16, kind="ExternalInput").ap()
    bT1 = nc.dram_tensor("bT1", [1, H], f16, kind="ExternalInput").ap()
    hTi = nc.dram_tensor("hTi", [L, KH, 128, BL], f16, kind="ExternalInput").ap()
    onesv = nc.dram_tensor("onesv", [1, COLS], f16, kind="ExternalInput").ap()
    outr = nc.dram_tensor("outr", [BL, Tb, H], f32, kind="ExternalOutput").ap()
    hlastT = nc.dram_tensor("hlastT", [L, KH, 128, BL], f32,
                            kind="ExternalOutput").ap()

    with TileContext(nc) as tc:
        with (
            tc.tile_pool(name="wpool", bufs=1) as wpool,
            tc.tile_pool(name="xtpool", bufs=1) as xtpool,
            tc.tile_pool(name="hpool", bufs=1) as hpool,
            tc.tile_pool(name="opool", bufs=1) as opool,
            tc.tile_pool(name="stage", bufs=4) as stage,
            tc.tile_pool(name="psum", bufs=1, space="PSUM") as pp,
        ):
            # ---- static tiles
            w0i = wpool.tile([128, KF, H], f16, tag="w0i")
            w0h = wpool.tile([128, KH, H], f16, tag="w0h")
            w1i = wpool.tile([128, KH, H], f16, tag="w1i")
            w1h = wpool.tile([128, KH, H], f16, tag="w1h")
            bt0 = wpool.tile([1, H], f16, tag="bt0")
            bt1 = wpool.tile([1, H], f16, tag="bt1")
            ones = wpool.tile([1, COLS], f16, tag="ones")
            hti = wpool.tile([128, L, KH, BL], f16, tag="hti")

            xT = xtpool.tile([128, KF, Tb * BL], f16, tag="xT")
            # history strips: [parity, kchunk, t, b]
            hist0 = hpool.tile([128, 2, KH, CS, BL], f16, tag="hist0")
            hist1 = hpool.tile([128, 2, KH, CS, BL], f16, tag="hist1")
            on16 = opool.tile([128, 4, H], f16, tag="on16")
            of32 = opool.tile([128, 4, H], f32, tag="of32")
            hlast = opool.tile([128, L, KH, BL], f32, tag="hlast")

            ps0 = pp.tile([128, M, 512], f32, tag="ps0")
            ps1 = pp.tile([128, M, 512], f32, tag="ps1")
            ps = [ps0, ps1]

            # ---- weight / const loads
            for k in range(KF):
                nc.sync.dma_start(out=w0i[:, k, :], in_=wihT0[k])
            for k in range(KH):
                nc.sync.dma_start(out=w0h[:, k, :], in_=whhT0[k])
                nc.sync.dma_start(out=w1i[:, k, :], in_=wihT1[k])
                nc.sync.dma_start(out=w1h[:, k, :], in_=whhT1[k])
            nc.sync.dma_start(out=bt0[:], in_=bT0[:])
            nc.sync.dma_start(out=bt1[:], in_=bT1[:])
            nc.sync.dma_start(out=ones[:], in_=onesv[:])
            for l in range(L):
                for k in range(KH):
                    nc.sync.dma_start(out=hti[:, l, k, :], in_=hTi[l, k])

            # ---- prepass: x -> fp16 -> transposed resident xT
            for blk in range(Tb // 16):
                xs = stage.tile([128, F], f32, tag="xs")
                xs16 = stage.tile([128, F], f16, tag="xs16")
                src = xr[:, ts(blk, 16), :].rearrange("b t f -> t b f")
                nc.sync.dma_start(out=xs[:], in_=src)
                if blk % 2 == 0:
                    nc.vector.tensor_copy(out=xs16[:], in_=xs[:])
                else:
                    nc.scalar.activation(xs16[:], xs[:], COPY)
                for fc in range(KF):
                    nc.sync.dma_start_transpose(
                        out=xT[:, fc, ds(blk * 128, 128)],
                        in_=xs16[:, ts(fc, 128)])

            whT = [w0h, w1h]
            hist = [hist0, hist1]
            bts = [bt0, bt1]

            def xproj(l, s, half, col0):
                """input projection + bias for layer l subchunk s into psum half"""
                off = half * COLS
                p = ps[l]
                for m in range(M):
                    nc.tensor.matmul(p[:, m, off:off + COLS],
                                     lhsT=bts[l][:, ts(m, 128)],
                                     rhs=ones[:, :COLS],
                                     start=True, stop=False,
                                     skip_group_check=True)
                if l == 0:
                    for k in range(KF):
                        rhs = xT[:, k, ds(col0, COLS)]
                        for m in range(M):
                            nc.tensor.matmul(p[:, m, off:off + COLS],
                                             lhsT=w0i[:, k, ts(m, 128)],
                                             rhs=rhs, start=False, stop=False,
                                             skip_group_check=True)
                else:
                    par = s % 2
                    for k in range(KH):
                        rhs = hist0[:, par, k, :, :]
                        for m in range(M):
                            nc.tensor.matmul(p[:, m, off:off + COLS],
                                             lhsT=w1i[:, k, ts(m, 128)],
                                             rhs=rhs, start=False, stop=False,
                                             skip_group_check=True)

            def recur_step(l, s, half, t, first):
                """one timestep of the recurrence: 16 MMs + 1 tanh"""
                par = s % 2
                p = ps[l]
                h = hist[l]
                off = half * COLS + t * BL
                for m in range(M):
                    for k in range(KH):
                        if t == 0:
                            rhs = (hti[:, l, k, :] if first
                                   else h[:, 1 - par, k, CS - 1, :])
                        else:
                            rhs = h[:, par, k, t - 1, :]
                        if col_tile:
                            for j in range(4):
                                nc.tensor.matmul(
                                    p[32 * j:32 * (j + 1), m, off:off + BL],
                                    lhsT=whT[l][:, k,
                                                ds(m * 128 + 32 * j, 32)],
                                    rhs=rhs, start=False,
                                    stop=(k == KH - 1),
                                    tile_position=(0, 32 * j),
                                    skip_group_check=True)
                        else:
                            nc.tensor.matmul(p[:, m, off:off + BL],
                                             lhsT=whT[l][:, k, ts(m, 128)],
                                             rhs=rhs, start=False,
                                             stop=(k == KH - 1),
                                             skip_group_check=True)
                nc.scalar.activation(h[:, par, :, t, :], p[:, :, off:off + BL],
                                     TANH)

            def recur_unit(l, s, half, first=False):
                return [lambda t=t: recur_step(l, s, half, t, first)
                        for t in range(CS)]

            def interleave(*units):
                for t in range(CS):
                    for u in units:
                        u[t]()

            def out_unit(s, t0):
                """ship layer-1 subchunk s (starting at global step t0) to DRAM"""
                par = s % 2
                for g in range(2):
                    slot = par * 2 + g
                    for c in range(KH):
                        nc.sync.dma_start_transpose(
                            out=on16[:, slot, ts(c, 128)],
                            in_=hist1[:, par, c, ts(g, 16), :])
                    if g == 0:
                        nc.vector.tensor_copy(out=of32[:, slot, :],
                                              in_=on16[:, slot, :])
                    else:
                        nc.scalar.activation(of32[:, slot, :], on16[:, slot, :],
                                             COPY)
                    dst = outr[:, ds(t0 + g * 16, 16), :].rearrange(
                        "b t h -> t b h")
                    nc.sync.dma_start(out=dst, in_=of32[:, slot, :])

            # ---- peel: L0(0); then L0(1) || L1(0)
            xproj(0, 0, 0, 0)
            interleave(recur_unit(0, 0, 0, first=True))
            xproj(0, 1, 1, COLS)
            xproj(1, 0, 0, None)
            interleave(recur_unit(0, 1, 1),
                       recur_unit(1, 0, 0, first=True))
            out_unit(0, 0)

            # ---- main loop: body i handles L0(2i),L1(2i-1),L0(2i+1),L1(2i)
            def main_loop():
                with tc.For_i(1, NB) as i:
                    col_even = i * (2 * COLS)          # xT col for s=2i
                    t_odd = i * (2 * CS) + (-CS)       # global t0 for s=2i-1
                    t_even = i * (2 * CS)              # global t0 for s=2i
                    xproj(0, 0, 0, col_even)           # s=2i  (parity 0)
                    xproj(1, 1, 1, None)               # s=2i-1 (parity 1)
                    interleave(recur_unit(0, 0, 0),    # L0(2i)
                               recur_unit(1, 1, 1))    # L1(2i-1)
                    out_unit(1, t_odd)
                    xproj(0, 1, 1, col_even + COLS)    # s=2i+1 (parity 1)
                    xproj(1, 0, 0, None)               # s=2i   (parity 0)
                    interleave(recur_unit(0, 1, 1),    # L0(2i+1)
                               recur_unit(1, 0, 0))    # L1(2i)
                    out_unit(0, t_even)

            if NB > 1:
                if repeat == 1:
                    main_loop()
                else:
                    with tc.For_i(0, repeat):
                        main_loop()

            # ---- tail: L1(S-1)
            xproj(1, S - 1, 1, None)
            interleave(recur_unit(1, S - 1, 1))
            out_unit(S - 1, Tb - CS)

            # ---- final hidden states
            fpar = (S - 1) % 2
            for l in range(L):
                nc.scalar.activation(hlast[:, l, :, :],
                                     hist[l][:, fpar, :, CS - 1, :], COPY)
                dst = hlastT[l].rearrange("c p b -> p c b")
                nc.sync.dma_start(out=dst, in_=hlast[:, l, :, :])

    nc.compile()
    return nc


_CACHE = {}
_RUNNERS = {}


class Runner:
    """jit-once PJRT runner for the compiled Bass module (8-core SPMD)."""

    def __init__(self, nc, n_cores=NCORES):
        import jax
        import numpy as _np
        from jax.sharding import Mesh, PartitionSpec
        from jax.experimental.shard_map import shard_map
        from concourse import bass2jax
        import concourse.mybir as mybir

        bass2jax.install_neuronx_cc_hook()
        self.n_cores = n_cores
        partition_name = (nc.partition_id_tensor.name
                          if nc.partition_id_tensor else None)
        in_names, out_names, out_avals, zero_shapes = [], [], [], []
        for alloc in nc.m.functions[0].allocations:
            if not isinstance(alloc, mybir.MemoryLocationSet):
                continue
            name = alloc.memorylocations[0].name
            if alloc.kind == "ExternalInput":
                if name != partition_name:
                    in_names.append(name)
            elif alloc.kind == "ExternalOutput":
                shape = tuple(alloc.tensor_shape)
                dtype = mybir.dt.np(alloc.dtype)
                out_names.append(name)
                out_avals.append(jax.core.ShapedArray(shape, dtype))
                zero_shapes.append((shape, dtype))
        self.n_params = len(in_names)
        self.in_names = list(in_names)
        self.out_names = out_names
        self.out_avals = out_avals
        self.zero_shapes = zero_shapes
        all_in = in_names + out_names + (
            [partition_name] if partition_name else [])

        def _body(*args):
            operands = list(args)
            if partition_name is not None:
                operands.append(bass2jax.partition_id_tensor())
            outs = bass2jax._bass_exec_p.bind(
                *operands,
                out_avals=tuple(out_avals),
                in_names=tuple(all_in),
                out_names=tuple(out_names),
                lowering_input_output_aliases=(),
                sim_require_finite=True,
                sim_require_nnan=True,
                nc=nc,
            )
            return tuple(outs)

        devices = jax.devices()[:n_cores]
        self.mesh = Mesh(_np.asarray(devices), ("core",))
        n_out = len(out_names)
        in_specs = (PartitionSpec("core"),) * (self.n_params + n_out)
        out_specs = (PartitionSpec("core"),) * n_out
        self.donate = tuple(range(self.n_params, self.n_params + n_out))
        self.fn = jax.jit(
            shard_map(_body, mesh=self.mesh, in_specs=in_specs,
                      out_specs=out_specs, check_rep=False),
            donate_argnums=self.donate, keep_unused=True)

    def concat_inputs(self, in_maps):
        return [np.concatenate([np.asarray(m[name]) for m in in_maps], 0)
                for name in self.in_names]

    def fresh_zeros(self):
        return [np.zeros((self.n_cores * s[0], *s[1:]), d)
                for s, d in self.zero_shapes]

    def call(self, concat_in, zeros):
        import jax
        outs = self.fn(*concat_in, *zeros)
        jax.block_until_ready(outs)
        return outs

    def run(self, in_maps):
        outs = self.call(self.concat_inputs(in_maps), self.fresh_zeros())
        res = []
        for c in range(self.n_cores):
            res.append({
                name: np.asarray(outs[i]).reshape(
                    self.n_cores, *self.out_avals[i].shape)[c]
                for i, name in enumerate(self.out_names)})
        return res


def get_runner(Tb=T):
    if Tb not in _RUNNERS:
        if Tb not in _CACHE:
            _CACHE[Tb] = build_nc(Tb)
        _RUNNERS[Tb] = Runner(_CACHE[Tb])
    return _RUNNERS[Tb]


def make_in_maps(x, h, Wih0, Whh0, b0, Wih1, Whh1, b1):
    com = _prep_weights(np.asarray(Wih0), np.asarray(Whh0), np.asarray(b0),
                        np.asarray(Wih1), np.asarray(Whh1), np.asarray(b1))
    h = np.asarray(h)
    in_maps = []
    for cidx in range(NCORES):
        bsl = slice(cidx * BL, (cidx + 1) * BL)
        hT = np.ascontiguousarray(
            h[:, bsl, :].transpose(0, 2, 1).astype(np.float16)
            .reshape(L, KH, 128, BL))
        m = dict(com)
        m["xr"] = np.ascontiguousarray(np.asarray(x)[bsl])
        m["hTi"] = hT
        in_maps.append(m)
    return in_maps


def postprocess(results, c):
    outs = np.concatenate([r["outr"] for r in results], 0)
    hparts = [r["hlastT"].transpose(0, 3, 1, 2).reshape(L, BL, H)
              for r in results]
    h_out = np.concatenate(hparts, 1)
    return outs, h_out, c


def _prep_weights(Wih0, Whh0, b0, Wih1, Whh1, b1):
    def t16(w, kc):
        # W [out, in] -> W.T [in, out] -> [kc, 128, out]
        return np.ascontiguousarray(
            w.T.astype(np.float16).reshape(kc, 128, w.shape[0]))
    com = {
        "wihT0": t16(Wih0, KF),
        "whhT0": t16(Whh0, KH),
        "wihT1": t16(Wih1, KH),
        "whhT1": t16(Whh1, KH),
        "bT0": b0.astype(np.float16).reshape(1, H),
        "bT1": b1.astype(np.float16).reshape(1, H),
        "onesv": np.ones((1, COLS), np.float16),
    }
    return com


def kernel(x, h, c, Wih0, Whh0, b0, Wih1, Whh1, b1, Tb=None):
    x = np.asarray(x)
    c = np.asarray(c)
    Tb = Tb or x.shape[1]
    runner = get_runner(Tb)
    in_maps = make_in_maps(x, h, Wih0, Whh0, b0, Wih1, Whh1, b1)
    results = runner.run(in_maps)
    return postprocess(results, c)


# revision 20
# speedup vs baseline: 94.8257x; 4.2356x over previous
"""Bass/Tile TRN2 kernel for 2-layer tanh RNN (B=64, T=2048, F=256, H=512).

Sharding: batch 64 -> 8 cores x 8 rows. Each core runs both layers for its
batch slice. Hidden state is kept transposed ([H partitions, batch cols]) so
each timestep is 16 fp16 128x128 matmuls accumulating into PSUM on top of a
precomputed input projection; one tanh ACT per step reads 4 PSUM banks and
writes the fp16 history strip that is also the next step's matmul operand.
Layer 1 subchunk s runs interleaved with layer 0 subchunk s+1 to hide
tanh/semaphore latency. x is cast+transposed on-chip via DMA XBAR transpose.
"""

import numpy as np

B, T, F, H, L = 64, 2048, 256, 512, 2
NCORES = 8
BL = B // NCORES            # 8 batch rows per core
CS = 32                     # timesteps per subchunk
COLS = CS * BL              # 256 psum cols per subchunk
KF = F // 128               # 2 k-chunks for layer-0 input proj
KH = H // 128               # 4 k-chunks for hidden
M = H // 128                # 4 m-chunks of output hidden

F16 = "float16"


def build_nc(Tb, repeat=1, col_tile=False):
    import concourse.bacc as bacc
    import concourse.mybir as mybir
    from concourse.tile import TileContext
    from concourse.bass import ds, ts

    f32 = mybir.dt.float32
    f16 = mybir.dt.float16
    TANH = mybir.ActivationFunctionType.Tanh
    COPY = mybir.ActivationFunctionType.Copy

    S = Tb // CS            # subchunks
    NB = S // 2             # loop bodies (2 subchunks each)
    assert Tb % (2 * CS) == 0

    nc = bacc.Bacc("TRN2", target_bir_lowering=False, debug=False,
                   num_devices=NCORES)

    xr = nc.dram_tensor("xr", [BL, Tb, F], f32, kind="ExternalInput").ap()
    wihT0 = nc.dram_tensor("wihT0", [KF, 128, H], f16, kind="ExternalInput").ap()
    whhT0 = nc.dram_tensor("whhT0", [KH, 128, H], f16, kind="ExternalInput").ap()
    wihT1 = nc.dram_tensor("wihT1", [KH, 128, H], f16, kind="ExternalInput").ap()
    whhT1 = nc.dram_tensor("whhT1", [KH, 128, H], f16, kind="ExternalInput").ap()
    bT0 = nc.dram_tensor("bT0", [1, H], f16, kind="ExternalInput").ap()
    bT1 = nc.dram_tensor("bT1", [1, H], f16, kind="ExternalInput").ap()
    hTi = nc.dram_tensor("hTi", [L, KH, 128, BL], f16, kind="ExternalInput").ap()
    onesv = nc.dram_tensor("onesv", [1, COLS], f16, kind="ExternalInput").ap()
    outr = nc.dram_tensor("outr", [BL, Tb, H], f32, kind="ExternalOutput").ap()
    hlastT = nc.dram_tensor("hlastT", [L, KH, 128, BL], f32,
                            kind="ExternalOutput").ap()

    with TileContext(nc) as tc:
        with (
            tc.tile_pool(name="wpool", bufs=1) as wpool,
            tc.tile_pool(name="xtpool", bufs=1) as xtpool,
            tc.tile_pool(name="hpool", bufs=1) as hpool,
            tc.tile_pool(name="opool", bufs=1) as opool,
            tc.tile_pool(name="stage", bufs=4) as stage,
            tc.tile_pool(name="psum", bufs=1, space="PSUM") as pp,
        ):
            # ---- static tiles
            w0i = wpool.tile([128, KF, H], f16, tag="w0i")
            w0h = wpool.tile([128, KH, H], f16, tag="w0h")
            w1i = wpool.tile([128, KH, H], f16, tag="w1i")
            w1h = wpool.tile([128, KH, H], f16, tag="w1h")
            bt0 = wpool.tile([1, H], f16, tag="bt0")
            bt1 = wpool.tile([1, H], f16, tag="bt1")
            ones = wpool.tile([1, COLS], f16, tag="ones")
            hti = wpool.tile([128, L, KH, BL], f16, tag="hti")

            xT = xtpool.tile([128, KF, Tb * BL], f16, tag="xT")
            # history strips: [parity, kchunk, t, b]
            hist0 = hpool.tile([128, 2, KH, CS, BL], f16, tag="hist0")
            hist1 = hpool.tile([128, 2, KH, CS, BL], f16, tag="hist1")
            on16 = opool.tile([128, 4, H], f16, tag="on16")
            of32 = opool.tile([128, 4, H], f32, tag="of32")
            hlast = opool.tile([128, L, KH, BL], f32, tag="hlast")

            ps0 = pp.tile([128, M, 512], f32, tag="ps0")
            ps1 = pp.tile([128, M, 512], f32, tag="ps1")
            ps = [ps0, ps1]

            # ---- weight / const loads
            for k in range(KF):
                nc.sync.dma_start(out=w0i[:, k, :], in_=wihT0[k])
            for k in range(KH):
                nc.sync.dma_start(out=w0h[:, k, :], in_=whhT0[k])
                nc.sync.dma_start(out=w1i[:, k, :], in_=wihT1[k])
                nc.sync.dma_start(out=w1h[:, k, :], in_=whhT1[k])
            nc.sync.dma_start(out=bt0[:], in_=bT0[:])
            nc.sync.dma_start(out=bt1[:], in_=bT1[:])
            nc.sync.dma_start(out=ones[:], in_=onesv[:])
            for l in range(L):
                for k in range(KH):
                    nc.sync.dma_start(out=hti[:, l, k, :], in_=hTi[l, k])

            # ---- prepass: x -> fp16 -> transposed resident xT
            for blk in range(Tb // 16):
                xs = stage.tile([128, F], f32, tag="xs")
                xs16 = stage.tile([128, F], f16, tag="xs16")
                src = xr[:, ts(blk, 16), :].rearrange("b t f -> t b f")
                nc.sync.dma_start(out=xs[:], in_=src)
                if blk % 2 == 0:
                    nc.vector.tensor_copy(out=xs16[:], in_=xs[:])
                else:
                    nc.scalar.activation(xs16[:], xs[:], COPY)
                for fc in range(KF):
                    nc.sync.dma_start_transpose(
                        out=xT[:, fc, ds(blk * 128, 128)],
                        in_=xs16[:, ts(fc, 128)])

            whT = [w0h, w1h]
            hist = [hist0, hist1]
            bts = [bt0, bt1]

            def xproj(l, s, half, col0):
                """input projection + bias for layer l subchunk s into psum half"""
                off = half * COLS
                p = ps[l]
                for m in range(M):
                    nc.tensor.matmul(p[:, m, off:off + COLS],
                                     lhsT=bts[l][:, ts(m, 128)],
                                     rhs=ones[:, :COLS],
                                     start=True, stop=False,
                                     skip_group_check=True)
                if l == 0:
                    for k in range(KF):
                        rhs = xT[:, k, ds(col0, COLS)]
                        for m in range(M):
                            nc.tensor.matmul(p[:, m, off:off + COLS],
                                             lhsT=w0i[:, k, ts(m, 128)],
                                             rhs=rhs, start=False, stop=False,
                                             skip_group_check=True)
                else:
                    par = s % 2
                    for k in range(KH):
                        rhs = hist0[:, par, k, :, :]
                        for m in range(M):
                            nc.tensor.matmul(p[:, m, off:off + COLS],
                                             lhsT=w1i[:, k, ts(m, 128)],
                                             rhs=rhs, start=False, stop=False,
                                             skip_group_check=True)

            def recur_step(l, s, half, t, first):
                """one timestep of the recurrence: 16 MMs + 1 tanh"""
                par = s % 2
                p = ps[l]
                h = hist[l]
                off = half * COLS + t * BL
                for m in range(M):
                    for k in range(KH):
                        if t == 0:
                            rhs = (hti[:, l, k, :] if first
                                   else h[:, 1 - par, k, CS - 1, :])
                        else:
                            rhs = h[:, par, k, t - 1, :]
                        if col_tile:
                            for j in range(4):
                                nc.tensor.matmul(
                                    p[32 * j:32 * (j + 1), m, off:off + BL],
                                    lhsT=whT[l][:, k,
                                                ds(m * 128 + 32 * j, 32)],
                                    rhs=rhs, start=False,
                                    stop=(k == KH - 1),
                                    tile_position=(0, 32 * j),
                                    skip_group_check=True)
                        else:
                            nc.tensor.matmul(p[:, m, off:off + BL],
                                             lhsT=whT[l][:, k, ts(m, 128)],
                                             rhs=rhs, start=False,
                                             stop=(k == KH - 1),
                                             skip_group_check=True)
                nc.scalar.activation(h[:, par, :, t, :], p[:, :, off:off + BL],
                                     TANH)

            def recur_unit(l, s, half, first=False):
                return [lambda t=t: recur_step(l, s, half, t, first)
                        for t in range(CS)]

            def interleave(*units):
                for t in range(CS):
                    for u in units:
                        u[t]()

            def out_unit(s, t0):
                """ship layer-1 subchunk s (starting at global step t0) to DRAM"""
                par = s % 2
                for g in range(2):
                    slot = par * 2 + g
                    for c in range(KH):
                        nc.sync.dma_start_transpose(
                            out=on16[:, slot, ts(c, 128)],
                            in_=hist1[:, par, c, ts(g, 16), :])
                    if g == 0:
                        nc.vector.tensor_copy(out=of32[:, slot, :],
                                              in_=on16[:, slot, :])
                    else:
                        nc.scalar.activation(of32[:, slot, :], on16[:, slot, :],
                                             COPY)
                    dst = outr[:, ds(t0 + g * 16, 16), :].rearrange(
                        "b t h -> t b h")
                    nc.sync.dma_start(out=dst, in_=of32[:, slot, :])

            # ---- peel: L0(0); then L0(1) || L1(0)
            xproj(0, 0, 0, 0)
            interleave(recur_unit(0, 0, 0, first=True))
            xproj(0, 1, 1, COLS)
            xproj(1, 0, 0, None)
            interleave(recur_unit(0, 1, 1),
                       recur_unit(1, 0, 0, first=True))
            out_unit(0, 0)

            # ---- main loop: body i handles L0(2i),L1(2i-1),L0(2i+1),L1(2i)
            def main_loop():
                with tc.For_i(1, NB,
                              hint_engines=(mybir.EngineType.PE,)) as i:
                    col_even = i * (2 * COLS)          # xT col for s=2i
                    t_odd = i * (2 * CS) + (-CS)       # global t0 for s=2i-1
                    t_even = i * (2 * CS)              # global t0 for s=2i
                    xproj(0, 0, 0, col_even)           # s=2i  (parity 0)
                    xproj(1, 1, 1, None)               # s=2i-1 (parity 1)
                    interleave(recur_unit(0, 0, 0),    # L0(2i)
                               recur_unit(1, 1, 1))    # L1(2i-1)
                    out_unit(1, t_odd)
                    xproj(0, 1, 1, col_even + COLS)    # s=2i+1 (parity 1)
                    xproj(1, 0, 0, None)               # s=2i   (parity 0)
                    interleave(recur_unit(0, 1, 1),    # L0(2i+1)
                               recur_unit(1, 0, 0))    # L1(2i)
                    out_unit(0, t_even)

            if NB > 1:
                if repeat == 1:
                    main_loop()
                else:
                    with tc.For_i(0, repeat):
                        main_loop()

            # ---- tail: L1(S-1)
            xproj(1, S - 1, 1, None)
            interleave(recur_unit(1, S - 1, 1))
            out_unit(S - 1, Tb - CS)

            # ---- final hidden states
            fpar = (S - 1) % 2
            for l in range(L):
                nc.scalar.activation(hlast[:, l, :, :],
                                     hist[l][:, fpar, :, CS - 1, :], COPY)
                dst = hlastT[l].rearrange("c p b -> p c b")
                nc.sync.dma_start(out=dst, in_=hlast[:, l, :, :])

    nc.compile()
    return nc


_CACHE = {}
_RUNNERS = {}


class Runner:
    """jit-once PJRT runner for the compiled Bass module (8-core SPMD)."""

    def __init__(self, nc, n_cores=NCORES):
        import jax
        import numpy as _np
        from jax.sharding import Mesh, PartitionSpec
        from jax.experimental.shard_map import shard_map
        from concourse import bass2jax
        import concourse.mybir as mybir

        bass2jax.install_neuronx_cc_hook()
        self.n_cores = n_cores
        partition_name = (nc.partition_id_tensor.name
                          if nc.partition_id_tensor else None)
        in_names, out_names, out_avals, zero_shapes = [], [], [], []
        for alloc in nc.m.functions[0].allocations:
            if not isinstance(alloc, mybir.MemoryLocationSet):
                continue
            name = alloc.memorylocations[0].name
            if alloc.kind == "ExternalInput":
                if name != partition_name:
                    in_names.append(name)
            elif alloc.kind == "ExternalOutput":
                shape = tuple(alloc.tensor_shape)
                dtype = mybir.dt.np(alloc.dtype)
                out_names.append(name)
                out_avals.append(jax.core.ShapedArray(shape, dtype))
                zero_shapes.append((shape, dtype))
        self.n_params = len(in_names)
        self.in_names = list(in_names)
        self.out_names = out_names
        self.out_avals = out_avals
        self.zero_shapes = zero_shapes
        all_in = in_names + out_names + (
            [partition_name] if partition_name else [])

        def _body(*args):
            operands = list(args)
            if partition_name is not None:
                operands.append(bass2jax.partition_id_tensor())
            outs = bass2jax._bass_exec_p.bind(
                *operands,
                out_avals=tuple(out_avals),
                in_names=tuple(all_in),
                out_names=tuple(out_names),
                lowering_input_output_aliases=(),
                sim_require_finite=True,
                sim_require_nnan=True,
                nc=nc,
            )
            return tuple(outs)

        devices = jax.devices()[:n_cores]
        self.mesh = Mesh(_np.asarray(devices), ("core",))
        n_out = len(out_names)
        in_specs = (PartitionSpec("core"),) * (self.n_params + n_out)
        out_specs = (PartitionSpec("core"),) * n_out
        self.donate = tuple(range(self.n_params, self.n_params + n_out))
        self.fn = jax.jit(
            shard_map(_body, mesh=self.mesh, in_specs=in_specs,
                      out_specs=out_specs, check_rep=False),
            donate_argnums=self.donate, keep_unused=True)

    def concat_inputs(self, in_maps):
        return [np.concatenate([np.asarray(m[name]) for m in in_maps], 0)
                for name in self.in_names]

    def fresh_zeros(self):
        return [np.zeros((self.n_cores * s[0], *s[1:]), d)
                for s, d in self.zero_shapes]

    def call(self, concat_in, zeros):
        import jax
        outs = self.fn(*concat_in, *zeros)
        jax.block_until_ready(outs)
        return outs

    def run(self, in_maps):
        outs = self.call(self.concat_inputs(in_maps), self.fresh_zeros())
        res = []
        for c in range(self.n_cores):
            res.append({
                name: np.asarray(outs[i]).reshape(
                    self.n_cores, *self.out_avals[i].shape)[c]
                for i, name in enumerate(self.out_names)})
        return res


def get_runner(Tb=T):
    if Tb not in _RUNNERS:
        if Tb not in _CACHE:
            _CACHE[Tb] = build_nc(Tb)
        _RUNNERS[Tb] = Runner(_CACHE[Tb])
    return _RUNNERS[Tb]


def make_in_maps(x, h, Wih0, Whh0, b0, Wih1, Whh1, b1):
    com = _prep_weights(np.asarray(Wih0), np.asarray(Whh0), np.asarray(b0),
                        np.asarray(Wih1), np.asarray(Whh1), np.asarray(b1))
    h = np.asarray(h)
    in_maps = []
    for cidx in range(NCORES):
        bsl = slice(cidx * BL, (cidx + 1) * BL)
        hT = np.ascontiguousarray(
            h[:, bsl, :].transpose(0, 2, 1).astype(np.float16)
            .reshape(L, KH, 128, BL))
        m = dict(com)
        m["xr"] = np.ascontiguousarray(np.asarray(x)[bsl])
        m["hTi"] = hT
        in_maps.append(m)
    return in_maps


def postprocess(results, c):
    outs = np.concatenate([r["outr"] for r in results], 0)
    hparts = [r["hlastT"].transpose(0, 3, 1, 2).reshape(L, BL, H)
              for r in results]
    h_out = np.concatenate(hparts, 1)
    return outs, h_out, c


def _prep_weights(Wih0, Whh0, b0, Wih1, Whh1, b1):
    def t16(w, kc):
        # W [out, in] -> W.T [in, out] -> [kc, 128, out]
        return np.ascontiguousarray(
            w.T.astype(np.float16).reshape(kc, 128, w.shape[0]))
    com = {
        "wihT0": t16(Wih0, KF),
        "whhT0": t16(Whh0, KH),
        "wihT1": t16(Wih1, KH),
        "whhT1": t16(Whh1, KH),
        "bT0": b0.astype(np.float16).reshape(1, H),
        "bT1": b1.astype(np.float16).reshape(1, H),
        "onesv": np.ones((1, COLS), np.float16),
    }
    return com


def kernel(x, h, c, Wih0, Whh0, b0, Wih1, Whh1, b1, Tb=None):
    x = np.asarray(x)
    c = np.asarray(c)
    Tb = Tb or x.shape[1]
    runner = get_runner(Tb)
    in_maps = make_in_maps(x, h, Wih0, Whh0, b0, Wih1, Whh1, b1)
    results = runner.run(in_maps)
    return postprocess(results, c)
